# revision 29
# baseline (speedup 1.0000x reference)
"""Fused multi-head attention (RoPE + GQA + softmax + o_proj) on 8 Trainium2 cores.

Sharding: core c handles batch b = c//2 and query-half qh = c%2 (1024 queries).
Each core computes full K/V for its batch, attention for its 1024 queries over
all 16 heads, and the output projection.

Precision strategy (keyed to the TRN2 cost model):
  - Q/K/V projections: fp8e4m3 DoubleRow matmuls with hi+lo residual
    compensation (comp3: (Wh+Wl)@(xh+xl) - Wl@xl), 0.75x the f32r/bf16
    cycle cost. hi/lo splits are precomputed on host (free).
  - scores / AV / o_proj: bf16 (pure-fp8 error exceeds the 2e-2 gate and
    compensation erases the speedup at contraction=128).
  - softmax: exp on ACT (bf16 out), denominator via a bf16 DVE add tree
    + a 1-row ones matmul on PE.

DoubleRow AP layout [128, 2, F] (slot dim in middle), interleave rules:
  x-like (moving in main):      slot0=hi, slot1=lo
  W-like (stationary in main):  slot0=lo, slot1=hi
  main (tile pair 2t,2t+1): stationary W[:, 2t:2t+2, 1, :], moving x[:, 2t:2t+2, 0, :]
  corr (tile t):            stationary W[:, t, :, :] (lo,hi), moving x[:, t, :, :] (hi,lo)
    => slot0 = W_lo . x_hi, slot1 = W_hi . x_lo  (the two cross terms)
"""

import sys

sys.path.insert(0, "/opt/trn_rl_repo")

import math

import numpy as np
import ml_dtypes

import concourse.mybir as mybir
import concourse.tile as tile
from concourse import bacc
from concourse.bass_utils import run_bass_kernel_spmd

P = 128
B, S, HID = 4, 2048, 2048
H, HKV, D = 16, 4, 128
SQ = S // 2
DC = HID // P  # 16
KVJ = HKV * D  # 512
REP = H // HKV  # 4
ROPE_THETA = 10000.0

F32 = mybir.dt.float32
BF16 = mybir.dt.bfloat16
FP8 = mybir.dt.float8e4
AL = mybir.AluOpType
AF = mybir.ActivationFunctionType
DR = mybir.MatmulPerfMode.DoubleRow

# fp8 / rope scales (host side)
S_X = 16.0
S_W = 1024.0
S_RQ = 128.0 / math.sqrt(D)  # folded into cos_q/sin_q (includes 1/sqrt(D))
S_RK = 16.0  # folded into cos_k/sin_k
EXP_SCALE = 1.0 / 2048.0  # descale scores psum (S_RQ*S_RK*sqrt(D) = 2048)
S_ATT = 512.0  # att fp8 scale, folded into the ones vector (den ones = 1/S_ATT)
S_WO = 1024.0

_CACHE = {}


def _comp3(stat, mov):
    """The 24 (lhsT, rhs) DR matmul operand pairs of a comp3 projection."""
    calls = []
    for t2 in range(DC // 2):
        calls.append((stat(t2, "main"), mov(t2, "main")))
    for t in range(DC):
        calls.append((stat(t, "corr"), mov(t, "corr")))
    return calls


def _emit(nc, psum_ap, calls, lo, hi):
    total = len(calls)
    for i in range(lo, hi):
        lhsT, rhs = calls[i]
        nc.tensor.matmul(
            psum_ap,
            lhsT=lhsT,
            rhs=rhs,
            start=(i == 0),
            stop=(i == total - 1),
            perf_mode=DR,
        )


def build_nc():
    if "nc" in _CACHE:
        return _CACHE["nc"]
    nc = bacc.Bacc("TRN2", target_bir_lowering=False)

    xc = nc.dram_tensor("xc", (P, DC, 2, S), FP8, kind="ExternalInput")
    xq = nc.dram_tensor("xq", (P, DC, 2, SQ), FP8, kind="ExternalInput")
    wqc = nc.dram_tensor("wqc", (H, P, DC, 2, D), FP8, kind="ExternalInput")
    wkc = nc.dram_tensor("wkc", (P, DC, 2, KVJ), FP8, kind="ExternalInput")
    wvc = nc.dram_tensor("wvc", (P, DC, 2, KVJ), FP8, kind="ExternalInput")
    woc = nc.dram_tensor("woc", (P, DC, 2, HID), FP8, kind="ExternalInput")
    cos_q = nc.dram_tensor("cos_q", (P, SQ), BF16, kind="ExternalInput")
    sin_q = nc.dram_tensor("sin_q", (P, SQ), BF16, kind="ExternalInput")
    cos_k = nc.dram_tensor("cos_k", (P, S), BF16, kind="ExternalInput")
    sin_k = nc.dram_tensor("sin_k", (P, S), BF16, kind="ExternalInput")
    pmat = nc.dram_tensor("pmat", (P, P), BF16, kind="ExternalInput")
    ones = nc.dram_tensor("ones", (P, 1), BF16, kind="ExternalInput")
    out = nc.dram_tensor("out", (SQ, HID), F32, kind="ExternalOutput")

    with tile.TileContext(nc) as tc:
        with (
            tc.tile_pool(name="consts", bufs=1) as consts,
            tc.tile_pool(name="kvp", bufs=1) as kvp,
            tc.tile_pool(name="qtab", bufs=1) as qtab,
            tc.tile_pool(name="wqp", bufs=3) as wqp,
        ):
            pm_t = consts.tile([P, P], BF16)
            nc.sync.dma_start(pm_t[:], pmat.ap())
            ones_t = consts.tile([P, 1], BF16)
            nc.sync.dma_start(ones_t[:], ones.ap())

            kt = kvp.tile([P, HKV, S], BF16)  # rope'd K^T, scale S_RK
            vt = kvp.tile([P, S // P, KVJ], BF16)  # V, true scale
            xq_t = kvp.tile([P, DC, 2, SQ], FP8)  # query-half x (hi/lo)

            def load_wq(h):
                w = wqp.tile([P, DC, 2, D], FP8, tag="wq", name=f"wq{h}")
                nc.sync.dma_start(w[:], wqc.ap()[h])
                return w

            # ---- Phase A: K/V projections (+ K rope) ----
            with (
                tc.tile_pool(name="xcp", bufs=1) as xcp,
                tc.tile_pool(name="wkvp", bufs=1) as wkvp,
                tc.tile_pool(name="ktab", bufs=1) as ktab,
                tc.tile_pool(name="workA", bufs=3) as workA,
                tc.tile_pool(name="ppKV", bufs=5, space="PSUM") as ppKV,
                tc.tile_pool(name="ppSw", bufs=3, space="PSUM") as ppSw,
            ):
                xc_t = xcp.tile([P, DC, 2, S], FP8)
                wkc_t = wkvp.tile([P, DC, 2, KVJ], FP8)
                wvc_t = wkvp.tile([P, DC, 2, KVJ], FP8)
                ck_t = ktab.tile([P, S], BF16)
                sk_t = ktab.tile([P, S], BF16)
                # DMA issue order ~ first-use order (HWDGE is FIFO per engine)
                nc.sync.dma_start(wkc_t[:, :, :, 0:P], wkc.ap()[:, :, :, 0:P])
                nc.scalar.dma_start(xc_t[:, :, :, 0:512], xc.ap()[:, :, :, 0:512])
                for kv in range(1, HKV):
                    nc.sync.dma_start(
                        wkc_t[:, :, :, kv * P : (kv + 1) * P],
                        wkc.ap()[:, :, :, kv * P : (kv + 1) * P],
                    )
                nc.sync.dma_start(ck_t[:], cos_k.ap())
                nc.sync.dma_start(sk_t[:], sin_k.ap())
                cq_t = qtab.tile([P, SQ], BF16)
                nc.sync.dma_start(cq_t[:], cos_q.ap())
                sq_t = qtab.tile([P, SQ], BF16)
                nc.sync.dma_start(sq_t[:], sin_q.ap())
                wq_pre = [load_wq(0)]
                for st in range(1, 4):
                    sl4 = slice(st * 512, (st + 1) * 512)
                    nc.scalar.dma_start(xc_t[:, :, :, sl4], xc.ap()[:, :, :, sl4])
                nc.sync.dma_start(wvc_t[:], wvc.ap())
                for dc in range(DC):
                    nc.scalar.dma_start(xq_t[:, dc], xq.ap()[:, dc])
                wq_pre.append(load_wq(1))

                for st in range(4):
                    sl = slice(st * 512, (st + 1) * 512)
                    for kv in range(HKV):
                        jsl = slice(kv * P, (kv + 1) * P)
                        pk = ppKV.tile([P, 512], F32, tag="pkv")
                        calls = _comp3(
                            lambda t, kind: (
                                wkc_t[:, 2 * t : 2 * t + 2, 1, jsl]
                                if kind == "main"
                                else wkc_t[:, t, :, jsl]
                            ),
                            lambda t, kind: (
                                xc_t[:, 2 * t : 2 * t + 2, 0, sl]
                                if kind == "main"
                                else xc_t[:, t, :, sl]
                            ),
                        )
                        _emit(nc, pk[:], calls, 0, 24)
                        kraw = workA.tile([P, 512], BF16, tag="kraw")
                        nc.scalar.activation(
                            kraw[:], pk[:], AF.Copy, scale=1.0 / (S_X * S_W)
                        )
                        sw = ppSw.tile([P, 512], F32, tag="ksw")
                        nc.tensor.matmul(
                            sw[:], lhsT=pm_t[:], rhs=kraw[:], start=True, stop=True
                        )
                        ta = workA.tile([P, 512], BF16, tag="kta")
                        nc.vector.tensor_tensor(ta[:], kraw[:], ck_t[:, sl], AL.mult)
                        tb = workA.tile([P, 512], BF16, tag="ktb")
                        nc.vector.tensor_tensor(tb[:], sw[:], sk_t[:, sl], AL.mult)
                        nc.vector.tensor_tensor(kt[:, kv, sl], ta[:], tb[:], AL.add)

                for pc in range(S // P):
                    psl = slice(pc * P, (pc + 1) * P)
                    pv = ppKV.tile([P, KVJ], F32, tag="pkv")
                    calls = _comp3(
                        lambda t, kind: (
                            xc_t[:, 2 * t : 2 * t + 2, 0, psl]
                            if kind == "main"
                            else xc_t[:, t, :, psl]
                        ),
                        lambda t, kind: (
                            wvc_t[:, 2 * t : 2 * t + 2, 1, :]
                            if kind == "main"
                            else wvc_t[:, t, :, :]
                        ),
                    )
                    _emit(nc, pv[:], calls, 0, 24)
                    nc.scalar.activation(
                        vt[:, pc, :], pv[:], AF.Copy, scale=1.0 / (S_X * S_W)
                    )

            with (
                tc.tile_pool(name="attp", bufs=1) as attp,
                tc.tile_pool(name="wop", bufs=2) as wop,
            ):
                attc = attp.tile([P, H, 2, SQ], FP8)  # att (d-part, h, hi/lo, q), scale S_ATT

                def load_wo(ot):
                    w = wop.tile([P, DC, 2, 512], FP8, tag="wo", name=f"wo{ot}")
                    nc.sync.dma_start(
                        w[:], woc.ap()[:, :, :, ot * 512 : (ot + 1) * 512]
                    )
                    return w

                # ---- Phase B: per head: Q proj + rope + attention ----
                with (
                    tc.tile_pool(name="qhp", bufs=2) as qhp,
                    tc.tile_pool(name="ptp", bufs=4) as ptp,
                    tc.tile_pool(name="workB", bufs=3) as workB,
                    tc.tile_pool(name="treeB", bufs=3) as treeB,
                    tc.tile_pool(name="ppQ", bufs=1, space="PSUM") as ppQ,
                    tc.tile_pool(name="ppSw2", bufs=1, space="PSUM") as ppSw2,
                    tc.tile_pool(name="ppSc", bufs=2, space="PSUM") as ppSc,
                    tc.tile_pool(name="ppAv", bufs=1, space="PSUM") as ppAv,
                    tc.tile_pool(name="ppDn", bufs=1, space="PSUM") as ppDn,
                ):
                    def qproj_calls(qt, wq_t):
                        qsl = slice(qt * 512, (qt + 1) * 512)
                        return _comp3(
                            lambda t, kind: (
                                wq_t[:, 2 * t : 2 * t + 2, 1, :]
                                if kind == "main"
                                else wq_t[:, t, :, :]
                            ),
                            lambda t, kind: (
                                xq_t[:, 2 * t : 2 * t + 2, 0, qsl]
                                if kind == "main"
                                else xq_t[:, t, :, qsl]
                            ),
                        )

                    def rope_q(qt, pq, qhead):
                        qsl = slice(qt * 512, (qt + 1) * 512)
                        qraw = workB.tile([P, 512], BF16, tag="qraw")
                        nc.scalar.activation(
                            qraw[:], pq[:], AF.Copy, scale=1.0 / (S_X * S_W)
                        )
                        sw = ppSw2.tile([P, 512], F32, tag="qsw")
                        nc.tensor.matmul(
                            sw[:], lhsT=pm_t[:], rhs=qraw[:], start=True, stop=True
                        )
                        ta = workB.tile([P, 512], BF16, tag="qta")
                        nc.vector.tensor_tensor(ta[:], qraw[:], cq_t[:, qsl], AL.mult)
                        tb = workB.tile([P, 512], BF16, tag="qtb")
                        nc.vector.tensor_tensor(tb[:], sw[:], sq_t[:, qsl], AL.mult)
                        nc.vector.tensor_tensor(qhead[:, qsl], ta[:], tb[:], AL.add)

                    # prologue: head 0 (wq 0,1 preloaded in phase A)
                    wq_cur = wq_pre[0]
                    wq_next = wq_pre[1]
                    qh_cur = qhp.tile([P, SQ], BF16, tag="qh", name="qh0")
                    for qt in range(2):
                        pq = ppQ.tile([P, 512], F32, tag="pq", name=f"pq0_{qt}")
                        _emit(nc, pq[:], qproj_calls(qt, wq_cur), 0, 24)
                        rope_q(qt, pq, qh_cur)

                    wo_sb = []
                    for h in range(H):
                        kv = h // REP
                        if h + 2 < H:
                            wq_after = load_wq(h + 2)
                        if h in (13, 14):
                            # prefetch o_proj weights during the phase B tail
                            wo_sb.append(load_wo(h - 13))
                        if h + 1 < H:
                            qh_next = qhp.tile([P, SQ], BF16, tag="qh", name=f"qh{h + 1}")
                        for qt in range(2):
                            qsl = slice(qt * 512, (qt + 1) * 512)
                            pq_next = None
                            qcalls = None
                            if h + 1 < H:
                                pq_next = ppQ.tile(
                                    [P, 512], F32, tag="pq", name=f"pq{h + 1}_{qt}"
                                )
                                qcalls = qproj_calls(qt, wq_next)
                            av = ppAv.tile([P, 512], F32, tag="av")
                            den = ppDn.tile([1, 512], F32, tag="den")
                            pts = []
                            s_tiles = []
                            for kp in range(8):
                                sc_ps = ppSc.tile([P, 2, 512], F32, tag="scps")
                                for i in range(2):
                                    kc = kp * 2 + i
                                    nc.tensor.matmul(
                                        sc_ps[:, i, :],
                                        lhsT=kt[:, kv, kc * P : (kc + 1) * P],
                                        rhs=qh_cur[:, qsl],
                                        start=True,
                                        stop=True,
                                    )
                                if kp >= 2:
                                    kcp = (kp - 2) * 2
                                    for i in range(2):
                                        nc.tensor.matmul(
                                            av[:],
                                            lhsT=vt[:, kcp + i, kv * P : (kv + 1) * P],
                                            rhs=pts[kp - 2][:, i, :],
                                            start=(kcp + i == 0),
                                            stop=False,
                                        )
                                if qcalls is not None:
                                    _emit(nc, pq_next[:], qcalls, 3 * kp, 3 * kp + 3)
                                pt = ptp.tile([P, 2, 512], BF16, tag="pt")
                                nc.scalar.activation(
                                    pt[:], sc_ps[:], AF.Exp, scale=EXP_SCALE
                                )
                                s_t = treeB.tile([P, 512], BF16, tag=f"s{kp % 2}")
                                nc.vector.tensor_tensor(
                                    s_t[:], pt[:, 0, :], pt[:, 1, :], AL.add
                                )
                                s_tiles.append(s_t)
                                if kp % 2 == 1:
                                    l2 = treeB.tile(
                                        [P, 512], BF16, tag=f"l2_{(kp // 2) % 2}"
                                    )
                                    nc.vector.tensor_tensor(
                                        l2[:], s_tiles[-2][:], s_tiles[-1][:], AL.add
                                    )
                                    s_tiles[-2:] = [l2]
                                if kp == 3 or kp == 7:
                                    l3 = treeB.tile([P, 512], BF16, tag=f"l3_{kp // 4}")
                                    nc.vector.tensor_tensor(
                                        l3[:], s_tiles[-2][:], s_tiles[-1][:], AL.add
                                    )
                                    s_tiles[-2:] = [l3]
                                pts.append(pt)
                            # rope for h+1 first: its ACT copy frees the pq
                            # psum bank before the next qt's qproj needs it
                            if pq_next is not None:
                                rope_q(qt, pq_next, qh_next)
                            for kp in (6, 7):
                                for i in range(2):
                                    nc.tensor.matmul(
                                        av[:],
                                        lhsT=vt[:, kp * 2 + i, kv * P : (kv + 1) * P],
                                        rhs=pts[kp][:, i, :],
                                        start=False,
                                        stop=(kp == 7 and i == 1),
                                    )
                            dfin = treeB.tile([P, 512], BF16, tag="dfin")
                            nc.vector.tensor_tensor(
                                dfin[:], s_tiles[0][:], s_tiles[1][:], AL.add
                            )
                            nc.tensor.matmul(
                                den[:], lhsT=ones_t[:], rhs=dfin[:], start=True, stop=True
                            )
                            # free the av psum bank early: copy to sbuf on DVE
                            av_sb = workB.tile([P, 512], F32, tag="avsb")
                            nc.vector.tensor_copy(av_sb[:], av[:])
                            r_row = workB.tile([1, 512], F32, tag="rrow")
                            nc.vector.reciprocal(r_row[:], den[:])
                            rb = workB.tile([P, 512], F32, tag="rb")
                            nc.gpsimd.partition_broadcast(rb[:], r_row[:])
                            att_bf = workB.tile([P, 512], BF16, tag="attbf")
                            nc.vector.tensor_tensor(
                                att_bf[:], av_sb[:], rb[:], AL.mult
                            )
                            nc.vector.tensor_copy(attc[:, h, 0, qsl], att_bf[:])
                            nc.vector.tensor_tensor(
                                attc[:, h, 1, qsl], att_bf[:], attc[:, h, 0, qsl], AL.subtract
                            )
                        if h + 1 < H:
                            wq_cur, qh_cur = wq_next, qh_next
                            if h + 2 < H:
                                wq_next = wq_after

                # ---- Phase C: o_proj (bf16) ----
                with (
                    tc.tile_pool(name="outp", bufs=4) as outp,
                    tc.tile_pool(name="ppO", bufs=6, space="PSUM") as ppO,
                ):
                    for ot in range(4):
                        wo_t = wo_sb[ot]
                        if ot + 2 < 4:
                            wo_sb.append(load_wo(ot + 2))
                        for qc in range(SQ // P):
                            qsl = slice(qc * P, (qc + 1) * P)
                            po = ppO.tile([P, 512], F32, tag="po")
                            calls = _comp3(
                                lambda t, kind: (
                                    attc[:, 2 * t : 2 * t + 2, 0, qsl]
                                    if kind == "main"
                                    else attc[:, t, :, qsl]
                                ),
                                lambda t, kind: (
                                    wo_t[:, 2 * t : 2 * t + 2, 1, :]
                                    if kind == "main"
                                    else wo_t[:, t, :, :]
                                ),
                            )
                            _emit(nc, po[:], calls, 0, 24)
                            o_t = outp.tile([P, 512], F32, tag="ot")
                            nc.scalar.activation(
                                o_t[:], po[:], AF.Copy, scale=1.0 / (S_ATT * S_WO)
                            )
                            nc.sync.dma_start(
                                out.ap()[qsl, ot * 512 : (ot + 1) * 512], o_t[:]
                            )

    nc.compile()
    _CACHE["nc"] = nc
    return nc


def _split_e4(a, scale):
    s = np.asarray(a, np.float32) * scale
    hi = s.astype(ml_dtypes.float8_e4m3)
    lo = (s - hi.astype(np.float32)).astype(ml_dtypes.float8_e4m3)
    return hi, lo


def _host_inputs(x, Wq, Wk, Wv, Wo):
    f32 = np.float32
    bf = ml_dtypes.bfloat16

    def w_style_scaled(w2d, inner, scale):
        # [HID, inner] -> [P, DC, 2, inner], slot0=lo slot1=hi
        arr = w2d.reshape(DC, P, inner).transpose(1, 0, 2)
        hi, lo = _split_e4(arr, scale)
        return np.ascontiguousarray(np.stack([lo, hi], axis=2))

    def w_style(w2d, inner):
        return w_style_scaled(w2d, inner, S_W)

    wkc = w_style(Wk.reshape(HID, KVJ), KVJ)
    wvc = w_style(Wv.reshape(HID, KVJ), KVJ)
    wq4 = Wq.reshape(DC, P, H, D).transpose(2, 1, 0, 3)  # [H, P, DC, D]
    hi, lo = _split_e4(wq4, S_W)
    wqc = np.ascontiguousarray(np.stack([lo, hi], axis=3))  # [H, P, DC, 2, D]
    woc = w_style_scaled(Wo.reshape(HID, HID), HID, S_WO)  # [P, DC, 2, HID]

    inv_ts = ROPE_THETA ** (-2.0 * np.arange(D // 2) / D)
    inv_full = np.concatenate([inv_ts, inv_ts])  # row d uses d%64
    pos_k = np.arange(S, dtype=np.float64)
    ang_k = inv_full[:, None] * pos_k[None, :]
    cos_k = (np.cos(ang_k) * S_RK).astype(bf)
    sin_k = (np.sin(ang_k) * S_RK).astype(bf)

    pmat = np.zeros((P, P), f32)  # lhsT: swap[i] = -q[i+64] (i<64), +q[i-64] (i>=64)
    for i in range(64):
        pmat[i + 64, i] = -1.0
        pmat[i, i + 64] = 1.0
    pmat = pmat.astype(bf)
    ones = (np.ones((P, 1), f32) / S_ATT).astype(bf)

    qtabs = []
    for qh in range(2):
        pos_q = np.arange(qh * SQ, (qh + 1) * SQ, dtype=np.float64)
        ang_q = inv_full[:, None] * pos_q[None, :]
        qtabs.append(
            ((np.cos(ang_q) * S_RQ).astype(bf), (np.sin(ang_q) * S_RQ).astype(bf))
        )

    xcs = []
    for b in range(B):
        xt = np.ascontiguousarray(x[b].T.reshape(DC, P, S).transpose(1, 0, 2))
        hi, lo = _split_e4(xt, S_X)
        xcs.append(np.ascontiguousarray(np.stack([hi, lo], axis=2)))  # x-style

    in_maps = []
    for c in range(8):
        b, qh = c // 2, c % 2
        cos_q, sin_q = qtabs[qh]
        in_maps.append(
            {
                "xc": xcs[b],
                "xq": np.ascontiguousarray(xcs[b][:, :, :, qh * SQ : (qh + 1) * SQ]),
                "wqc": wqc,
                "wkc": wkc,
                "wvc": wvc,
                "woc": woc,
                "cos_q": cos_q,
                "sin_q": sin_q,
                "cos_k": cos_k,
                "sin_k": sin_k,
                "pmat": pmat,
                "ones": ones,
            }
        )
    return in_maps


def kernel(x, Wq, Wk, Wv, Wo, _trace=False):
    x, Wq, Wk, Wv, Wo = (np.asarray(a, dtype=np.float32) for a in (x, Wq, Wk, Wv, Wo))
    nc = build_nc()
    in_maps = _host_inputs(x, Wq, Wk, Wv, Wo)
    res = run_bass_kernel_spmd(nc, in_maps, core_ids=list(range(8)), trace=_trace)
    out = np.empty((B, S, HID), np.float32)
    for c in range(8):
        b, qh = c // 2, c % 2
        out[b, qh * SQ : (qh + 1) * SQ] = res.results[c]["out"]
    if _trace:
        kernel.last_results = res
    return out


# revision 30
# speedup vs baseline: 1.0558x; 1.0558x over previous
"""Fused multi-head attention (RoPE + GQA + softmax + o_proj) on 8 Trainium2 cores.

Sharding: core c handles batch b = c//2 and query-half qh = c%2 (1024 queries).
Each core computes full K/V for its batch, attention for its 1024 queries over
all 16 heads, and the output projection.

Precision strategy (keyed to the TRN2 cost model):
  - Q/K/V projections: fp8e4m3 DoubleRow matmuls with hi+lo residual
    compensation (comp3: (Wh+Wl)@(xh+xl) - Wl@xl), 0.75x the f32r/bf16
    cycle cost. hi/lo splits are precomputed on host (free).
  - scores / AV / o_proj: bf16 (pure-fp8 error exceeds the 2e-2 gate and
    compensation erases the speedup at contraction=128).
  - softmax: exp on ACT (bf16 out), denominator via a bf16 DVE add tree
    + a 1-row ones matmul on PE.

DoubleRow AP layout [128, 2, F] (slot dim in middle), interleave rules:
  x-like (moving in main):      slot0=hi, slot1=lo
  W-like (stationary in main):  slot0=lo, slot1=hi
  main (tile pair 2t,2t+1): stationary W[:, 2t:2t+2, 1, :], moving x[:, 2t:2t+2, 0, :]
  corr (tile t):            stationary W[:, t, :, :] (lo,hi), moving x[:, t, :, :] (hi,lo)
    => slot0 = W_lo . x_hi, slot1 = W_hi . x_lo  (the two cross terms)
"""

import sys

sys.path.insert(0, "/opt/trn_rl_repo")

import math

import numpy as np
import ml_dtypes

import concourse.mybir as mybir
import concourse.tile as tile
from concourse import bacc
from concourse.bass_utils import run_bass_kernel_spmd

P = 128
B, S, HID = 4, 2048, 2048
H, HKV, D = 16, 4, 128
SQ = S // 2
DC = HID // P  # 16
KVJ = HKV * D  # 512
REP = H // HKV  # 4
ROPE_THETA = 10000.0

F32 = mybir.dt.float32
BF16 = mybir.dt.bfloat16
FP8 = mybir.dt.float8e4
AL = mybir.AluOpType
AF = mybir.ActivationFunctionType
DR = mybir.MatmulPerfMode.DoubleRow

# fp8 / rope scales (host side)
S_X = 16.0
S_W = 1024.0
S_RQ = 128.0 / math.sqrt(D)  # folded into cos_q/sin_q (includes 1/sqrt(D))
S_RK = 16.0  # folded into cos_k/sin_k
EXP_SCALE = 1.0 / 2048.0  # descale scores psum (S_RQ*S_RK*sqrt(D) = 2048)
S_ATT = 512.0  # att fp8 scale, folded into the ones vector (den ones = 1/S_ATT)
S_WO = 1024.0

_CACHE = {}


def _comp3(stat, mov):
    """The 24 (lhsT, rhs) DR matmul operand pairs of a comp3 projection."""
    calls = []
    for t2 in range(DC // 2):
        calls.append((stat(t2, "main"), mov(t2, "main")))
    for t in range(DC):
        calls.append((stat(t, "corr"), mov(t, "corr")))
    return calls


def _emit(nc, psum_ap, calls, lo, hi):
    total = len(calls)
    for i in range(lo, hi):
        lhsT, rhs = calls[i]
        nc.tensor.matmul(
            psum_ap,
            lhsT=lhsT,
            rhs=rhs,
            start=(i == 0),
            stop=(i == total - 1),
            perf_mode=DR,
        )


def build_nc():
    if "nc" in _CACHE:
        return _CACHE["nc"]
    nc = bacc.Bacc("TRN2", target_bir_lowering=False)

    xc = nc.dram_tensor("xc", (P, DC, 2, S), FP8, kind="ExternalInput")
    xq = nc.dram_tensor("xq", (P, DC, 2, SQ), FP8, kind="ExternalInput")
    wqc = nc.dram_tensor("wqc", (H, P, DC, 2, D), FP8, kind="ExternalInput")
    wkc = nc.dram_tensor("wkc", (P, DC, 2, KVJ), FP8, kind="ExternalInput")
    wvc = nc.dram_tensor("wvc", (P, DC, 2, KVJ), FP8, kind="ExternalInput")
    woc = nc.dram_tensor("woc", (P, DC, 2, HID), FP8, kind="ExternalInput")
    cos_q = nc.dram_tensor("cos_q", (P, SQ), BF16, kind="ExternalInput")
    sin_q = nc.dram_tensor("sin_q", (P, SQ), BF16, kind="ExternalInput")
    cos_k = nc.dram_tensor("cos_k", (P, S), BF16, kind="ExternalInput")
    sin_k = nc.dram_tensor("sin_k", (P, S), BF16, kind="ExternalInput")
    pmat = nc.dram_tensor("pmat", (P, P), BF16, kind="ExternalInput")
    ones = nc.dram_tensor("ones", (P, 1), BF16, kind="ExternalInput")
    out = nc.dram_tensor("out", (SQ, HID), F32, kind="ExternalOutput")

    with tile.TileContext(nc) as tc:
        with (
            tc.tile_pool(name="consts", bufs=1) as consts,
            tc.tile_pool(name="kvp", bufs=1) as kvp,
            tc.tile_pool(name="qtab", bufs=1) as qtab,
            tc.tile_pool(name="wqp", bufs=3) as wqp,
        ):
            pm_t = consts.tile([P, P], BF16)
            nc.sync.dma_start(pm_t[:], pmat.ap())
            ones_t = consts.tile([P, 1], BF16)
            nc.sync.dma_start(ones_t[:], ones.ap())

            kt = kvp.tile([P, HKV, S], BF16)  # rope'd K^T, scale S_RK
            vt = kvp.tile([P, S // P, KVJ], BF16)  # V, true scale
            xq_t = kvp.tile([P, DC, 2, SQ], FP8)  # query-half x (hi/lo)

            def load_wq(h):
                w = wqp.tile([P, DC, 2, D], FP8, tag="wq", name=f"wq{h}")
                nc.sync.dma_start(w[:], wqc.ap()[h])
                return w

            # ---- Phase A: K/V projections (+ K rope) ----
            with (
                tc.tile_pool(name="xcp", bufs=1) as xcp,
                tc.tile_pool(name="wkvp", bufs=1) as wkvp,
                tc.tile_pool(name="ktab", bufs=1) as ktab,
                tc.tile_pool(name="workA", bufs=3) as workA,
                tc.tile_pool(name="ppKV", bufs=5, space="PSUM") as ppKV,
                tc.tile_pool(name="ppSw", bufs=3, space="PSUM") as ppSw,
            ):
                xc_t = xcp.tile([P, DC, 2, S], FP8)
                wkc_t = wkvp.tile([P, DC, 2, KVJ], FP8)
                wvc_t = wkvp.tile([P, DC, 2, KVJ], FP8)
                ck_t = ktab.tile([P, S], BF16)
                sk_t = ktab.tile([P, S], BF16)
                # DMA issue order ~ first-use order (HWDGE is FIFO per engine)
                nc.sync.dma_start(wkc_t[:, :, :, 0:P], wkc.ap()[:, :, :, 0:P])
                nc.sync.dma_start(xc_t[:, :, :, 0:512], xc.ap()[:, :, :, 0:512])
                for kv in range(1, HKV):
                    nc.sync.dma_start(
                        wkc_t[:, :, :, kv * P : (kv + 1) * P],
                        wkc.ap()[:, :, :, kv * P : (kv + 1) * P],
                    )
                nc.sync.dma_start(ck_t[:], cos_k.ap())
                nc.sync.dma_start(sk_t[:], sin_k.ap())
                cq_t = qtab.tile([P, SQ], BF16)
                nc.sync.dma_start(cq_t[:], cos_q.ap())
                sq_t = qtab.tile([P, SQ], BF16)
                nc.sync.dma_start(sq_t[:], sin_q.ap())
                wq_pre = [load_wq(0)]
                for st in range(1, 4):
                    sl4 = slice(st * 512, (st + 1) * 512)
                    nc.sync.dma_start(xc_t[:, :, :, sl4], xc.ap()[:, :, :, sl4])
                nc.sync.dma_start(wvc_t[:], wvc.ap())
                for dc in range(DC):
                    nc.sync.dma_start(xq_t[:, dc], xq.ap()[:, dc])
                wq_pre.append(load_wq(1))

                for st in range(4):
                    sl = slice(st * 512, (st + 1) * 512)
                    for kv in range(HKV):
                        jsl = slice(kv * P, (kv + 1) * P)
                        pk = ppKV.tile([P, 512], F32, tag="pkv")
                        calls = _comp3(
                            lambda t, kind: (
                                wkc_t[:, 2 * t : 2 * t + 2, 1, jsl]
                                if kind == "main"
                                else wkc_t[:, t, :, jsl]
                            ),
                            lambda t, kind: (
                                xc_t[:, 2 * t : 2 * t + 2, 0, sl]
                                if kind == "main"
                                else xc_t[:, t, :, sl]
                            ),
                        )
                        _emit(nc, pk[:], calls, 0, 24)
                        kraw = workA.tile([P, 512], BF16, tag="kraw")
                        nc.scalar.activation(
                            kraw[:], pk[:], AF.Copy, scale=1.0 / (S_X * S_W)
                        )
                        sw = ppSw.tile([P, 512], F32, tag="ksw")
                        nc.tensor.matmul(
                            sw[:], lhsT=pm_t[:], rhs=kraw[:], start=True, stop=True
                        )
                        ta = workA.tile([P, 512], BF16, tag="kta")
                        nc.vector.tensor_tensor(ta[:], kraw[:], ck_t[:, sl], AL.mult)
                        tb = workA.tile([P, 512], BF16, tag="ktb")
                        nc.vector.tensor_tensor(tb[:], sw[:], sk_t[:, sl], AL.mult)
                        nc.vector.tensor_tensor(kt[:, kv, sl], ta[:], tb[:], AL.add)

                for pc in range(S // P):
                    psl = slice(pc * P, (pc + 1) * P)
                    pv = ppKV.tile([P, KVJ], F32, tag="pkv")
                    calls = _comp3(
                        lambda t, kind: (
                            xc_t[:, 2 * t : 2 * t + 2, 0, psl]
                            if kind == "main"
                            else xc_t[:, t, :, psl]
                        ),
                        lambda t, kind: (
                            wvc_t[:, 2 * t : 2 * t + 2, 1, :]
                            if kind == "main"
                            else wvc_t[:, t, :, :]
                        ),
                    )
                    _emit(nc, pv[:], calls, 0, 24)
                    nc.scalar.activation(
                        vt[:, pc, :], pv[:], AF.Copy, scale=1.0 / (S_X * S_W)
                    )

            with (
                tc.tile_pool(name="attp", bufs=1) as attp,
                tc.tile_pool(name="wop", bufs=2) as wop,
            ):
                attc = attp.tile([P, H, 2, SQ], FP8)  # att (d-part, h, hi/lo, q), scale S_ATT

                def load_wo(ot):
                    w = wop.tile([P, DC, 2, 512], FP8, tag="wo", name=f"wo{ot}")
                    nc.sync.dma_start(
                        w[:], woc.ap()[:, :, :, ot * 512 : (ot + 1) * 512]
                    )
                    return w

                # ---- Phase B: per head: Q proj + rope + attention ----
                with (
                    tc.tile_pool(name="qhp", bufs=2) as qhp,
                    tc.tile_pool(name="ptp", bufs=4) as ptp,
                    tc.tile_pool(name="workB", bufs=3) as workB,
                    tc.tile_pool(name="treeB", bufs=3) as treeB,
                    tc.tile_pool(name="ppQ", bufs=1, space="PSUM") as ppQ,
                    tc.tile_pool(name="ppSw2", bufs=1, space="PSUM") as ppSw2,
                    tc.tile_pool(name="ppSc", bufs=2, space="PSUM") as ppSc,
                    tc.tile_pool(name="ppAv", bufs=1, space="PSUM") as ppAv,
                    tc.tile_pool(name="ppDn", bufs=1, space="PSUM") as ppDn,
                ):
                    def qproj_calls(qt, wq_t):
                        qsl = slice(qt * 512, (qt + 1) * 512)
                        return _comp3(
                            lambda t, kind: (
                                wq_t[:, 2 * t : 2 * t + 2, 1, :]
                                if kind == "main"
                                else wq_t[:, t, :, :]
                            ),
                            lambda t, kind: (
                                xq_t[:, 2 * t : 2 * t + 2, 0, qsl]
                                if kind == "main"
                                else xq_t[:, t, :, qsl]
                            ),
                        )

                    def rope_q(qt, pq, qhead):
                        qsl = slice(qt * 512, (qt + 1) * 512)
                        qraw = workB.tile([P, 512], BF16, tag="qraw")
                        nc.scalar.activation(
                            qraw[:], pq[:], AF.Copy, scale=1.0 / (S_X * S_W)
                        )
                        sw = ppSw2.tile([P, 512], F32, tag="qsw")
                        nc.tensor.matmul(
                            sw[:], lhsT=pm_t[:], rhs=qraw[:], start=True, stop=True
                        )
                        ta = workB.tile([P, 512], BF16, tag="qta")
                        nc.vector.tensor_tensor(ta[:], qraw[:], cq_t[:, qsl], AL.mult)
                        tb = workB.tile([P, 512], BF16, tag="qtb")
                        nc.vector.tensor_tensor(tb[:], sw[:], sq_t[:, qsl], AL.mult)
                        nc.vector.tensor_tensor(qhead[:, qsl], ta[:], tb[:], AL.add)

                    # prologue: head 0 (wq 0,1 preloaded in phase A)
                    wq_cur = wq_pre[0]
                    wq_next = wq_pre[1]
                    qh_cur = qhp.tile([P, SQ], BF16, tag="qh", name="qh0")
                    for qt in range(2):
                        pq = ppQ.tile([P, 512], F32, tag="pq", name=f"pq0_{qt}")
                        _emit(nc, pq[:], qproj_calls(qt, wq_cur), 0, 24)
                        rope_q(qt, pq, qh_cur)

                    wo_sb = []
                    for h in range(H):
                        kv = h // REP
                        if h + 2 < H:
                            wq_after = load_wq(h + 2)
                        if h in (13, 14):
                            # prefetch o_proj weights during the phase B tail
                            wo_sb.append(load_wo(h - 13))
                        if h + 1 < H:
                            qh_next = qhp.tile([P, SQ], BF16, tag="qh", name=f"qh{h + 1}")
                        for qt in range(2):
                            qsl = slice(qt * 512, (qt + 1) * 512)
                            pq_next = None
                            qcalls = None
                            if h + 1 < H:
                                pq_next = ppQ.tile(
                                    [P, 512], F32, tag="pq", name=f"pq{h + 1}_{qt}"
                                )
                                qcalls = qproj_calls(qt, wq_next)
                            av = ppAv.tile([P, 512], F32, tag="av")
                            den = ppDn.tile([1, 512], F32, tag="den")
                            pts = []
                            s_tiles = []
                            for kp in range(8):
                                sc_ps = ppSc.tile([P, 2, 512], F32, tag="scps")
                                for i in range(2):
                                    kc = kp * 2 + i
                                    nc.tensor.matmul(
                                        sc_ps[:, i, :],
                                        lhsT=kt[:, kv, kc * P : (kc + 1) * P],
                                        rhs=qh_cur[:, qsl],
                                        start=True,
                                        stop=True,
                                    )
                                if kp >= 2:
                                    kcp = (kp - 2) * 2
                                    for i in range(2):
                                        nc.tensor.matmul(
                                            av[:],
                                            lhsT=vt[:, kcp + i, kv * P : (kv + 1) * P],
                                            rhs=pts[kp - 2][:, i, :],
                                            start=(kcp + i == 0),
                                            stop=False,
                                        )
                                if qcalls is not None:
                                    _emit(nc, pq_next[:], qcalls, 3 * kp, 3 * kp + 3)
                                pt = ptp.tile([P, 2, 512], BF16, tag="pt")
                                nc.scalar.activation(
                                    pt[:], sc_ps[:], AF.Exp, scale=EXP_SCALE
                                )
                                s_t = treeB.tile([P, 512], BF16, tag=f"s{kp % 2}")
                                nc.vector.tensor_tensor(
                                    s_t[:], pt[:, 0, :], pt[:, 1, :], AL.add
                                )
                                s_tiles.append(s_t)
                                if kp % 2 == 1:
                                    l2 = treeB.tile(
                                        [P, 512], BF16, tag=f"l2_{(kp // 2) % 2}"
                                    )
                                    nc.vector.tensor_tensor(
                                        l2[:], s_tiles[-2][:], s_tiles[-1][:], AL.add
                                    )
                                    s_tiles[-2:] = [l2]
                                if kp == 3 or kp == 7:
                                    l3 = treeB.tile([P, 512], BF16, tag=f"l3_{kp // 4}")
                                    nc.vector.tensor_tensor(
                                        l3[:], s_tiles[-2][:], s_tiles[-1][:], AL.add
                                    )
                                    s_tiles[-2:] = [l3]
                                pts.append(pt)
                            # rope for h+1 first: its ACT copy frees the pq
                            # psum bank before the next qt's qproj needs it
                            if pq_next is not None:
                                rope_q(qt, pq_next, qh_next)
                            for kp in (6, 7):
                                for i in range(2):
                                    nc.tensor.matmul(
                                        av[:],
                                        lhsT=vt[:, kp * 2 + i, kv * P : (kv + 1) * P],
                                        rhs=pts[kp][:, i, :],
                                        start=False,
                                        stop=(kp == 7 and i == 1),
                                    )
                            dfin = treeB.tile([P, 512], BF16, tag="dfin")
                            nc.vector.tensor_tensor(
                                dfin[:], s_tiles[0][:], s_tiles[1][:], AL.add
                            )
                            nc.tensor.matmul(
                                den[:], lhsT=ones_t[:], rhs=dfin[:], start=True, stop=True
                            )
                            # free the av psum bank early: copy to sbuf on DVE
                            av_sb = workB.tile([P, 512], F32, tag="avsb")
                            nc.vector.tensor_copy(av_sb[:], av[:])
                            r_row = workB.tile([1, 512], F32, tag="rrow")
                            nc.vector.reciprocal(r_row[:], den[:])
                            rb = workB.tile([P, 512], F32, tag="rb")
                            nc.gpsimd.partition_broadcast(rb[:], r_row[:])
                            att_bf = workB.tile([P, 512], BF16, tag="attbf")
                            nc.vector.tensor_tensor(
                                att_bf[:], av_sb[:], rb[:], AL.mult
                            )
                            nc.vector.tensor_copy(attc[:, h, 0, qsl], att_bf[:])
                            nc.vector.tensor_tensor(
                                attc[:, h, 1, qsl], att_bf[:], attc[:, h, 0, qsl], AL.subtract
                            )
                        if h + 1 < H:
                            wq_cur, qh_cur = wq_next, qh_next
                            if h + 2 < H:
                                wq_next = wq_after

                # ---- Phase C: o_proj (bf16) ----
                with (
                    tc.tile_pool(name="outp", bufs=4) as outp,
                    tc.tile_pool(name="ppO", bufs=6, space="PSUM") as ppO,
                ):
                    for ot in range(4):
                        wo_t = wo_sb[ot]
                        if ot + 2 < 4:
                            wo_sb.append(load_wo(ot + 2))
                        for qc in range(SQ // P):
                            qsl = slice(qc * P, (qc + 1) * P)
                            po = ppO.tile([P, 512], F32, tag="po")
                            calls = _comp3(
                                lambda t, kind: (
                                    attc[:, 2 * t : 2 * t + 2, 0, qsl]
                                    if kind == "main"
                                    else attc[:, t, :, qsl]
                                ),
                                lambda t, kind: (
                                    wo_t[:, 2 * t : 2 * t + 2, 1, :]
                                    if kind == "main"
                                    else wo_t[:, t, :, :]
                                ),
                            )
                            _emit(nc, po[:], calls, 0, 24)
                            o_t = outp.tile([P, 512], F32, tag="ot")
                            nc.scalar.activation(
                                o_t[:], po[:], AF.Copy, scale=1.0 / (S_ATT * S_WO)
                            )
                            nc.sync.dma_start(
                                out.ap()[qsl, ot * 512 : (ot + 1) * 512], o_t[:]
                            )

    nc.compile()
    _CACHE["nc"] = nc
    return nc


def _split_e4(a, scale):
    s = np.asarray(a, np.float32) * scale
    hi = s.astype(ml_dtypes.float8_e4m3)
    lo = (s - hi.astype(np.float32)).astype(ml_dtypes.float8_e4m3)
    return hi, lo


def _host_inputs(x, Wq, Wk, Wv, Wo):
    f32 = np.float32
    bf = ml_dtypes.bfloat16

    def w_style_scaled(w2d, inner, scale):
        # [HID, inner] -> [P, DC, 2, inner], slot0=lo slot1=hi
        arr = w2d.reshape(DC, P, inner).transpose(1, 0, 2)
        hi, lo = _split_e4(arr, scale)
        return np.ascontiguousarray(np.stack([lo, hi], axis=2))

    def w_style(w2d, inner):
        return w_style_scaled(w2d, inner, S_W)

    wkc = w_style(Wk.reshape(HID, KVJ), KVJ)
    wvc = w_style(Wv.reshape(HID, KVJ), KVJ)
    wq4 = Wq.reshape(DC, P, H, D).transpose(2, 1, 0, 3)  # [H, P, DC, D]
    hi, lo = _split_e4(wq4, S_W)
    wqc = np.ascontiguousarray(np.stack([lo, hi], axis=3))  # [H, P, DC, 2, D]
    woc = w_style_scaled(Wo.reshape(HID, HID), HID, S_WO)  # [P, DC, 2, HID]

    inv_ts = ROPE_THETA ** (-2.0 * np.arange(D // 2) / D)
    inv_full = np.concatenate([inv_ts, inv_ts])  # row d uses d%64
    pos_k = np.arange(S, dtype=np.float64)
    ang_k = inv_full[:, None] * pos_k[None, :]
    cos_k = (np.cos(ang_k) * S_RK).astype(bf)
    sin_k = (np.sin(ang_k) * S_RK).astype(bf)

    pmat = np.zeros((P, P), f32)  # lhsT: swap[i] = -q[i+64] (i<64), +q[i-64] (i>=64)
    for i in range(64):
        pmat[i + 64, i] = -1.0
        pmat[i, i + 64] = 1.0
    pmat = pmat.astype(bf)
    ones = (np.ones((P, 1), f32) / S_ATT).astype(bf)

    qtabs = []
    for qh in range(2):
        pos_q = np.arange(qh * SQ, (qh + 1) * SQ, dtype=np.float64)
        ang_q = inv_full[:, None] * pos_q[None, :]
        qtabs.append(
            ((np.cos(ang_q) * S_RQ).astype(bf), (np.sin(ang_q) * S_RQ).astype(bf))
        )

    xcs = []
    for b in range(B):
        xt = np.ascontiguousarray(x[b].T.reshape(DC, P, S).transpose(1, 0, 2))
        hi, lo = _split_e4(xt, S_X)
        xcs.append(np.ascontiguousarray(np.stack([hi, lo], axis=2)))  # x-style

    in_maps = []
    for c in range(8):
        b, qh = c // 2, c % 2
        cos_q, sin_q = qtabs[qh]
        in_maps.append(
            {
                "xc": xcs[b],
                "xq": np.ascontiguousarray(xcs[b][:, :, :, qh * SQ : (qh + 1) * SQ]),
                "wqc": wqc,
                "wkc": wkc,
                "wvc": wvc,
                "woc": woc,
                "cos_q": cos_q,
                "sin_q": sin_q,
                "cos_k": cos_k,
                "sin_k": sin_k,
                "pmat": pmat,
                "ones": ones,
            }
        )
    return in_maps


def kernel(x, Wq, Wk, Wv, Wo, _trace=False):
    x, Wq, Wk, Wv, Wo = (np.asarray(a, dtype=np.float32) for a in (x, Wq, Wk, Wv, Wo))
    nc = build_nc()
    in_maps = _host_inputs(x, Wq, Wk, Wv, Wo)
    res = run_bass_kernel_spmd(nc, in_maps, core_ids=list(range(8)), trace=_trace)
    out = np.empty((B, S, HID), np.float32)
    for c in range(8):
        b, qh = c // 2, c % 2
        out[b, qh * SQ : (qh + 1) * SQ] = res.results[c]["out"]
    if _trace:
        kernel.last_results = res
    return out
